# revision 7
# baseline (speedup 1.0000x reference)
"""Trainium2 Bass kernel for nn_DialogueGCNModel (DialogueGCN forward).

Strategy (data-parallel over dialogues, 4 dialogues per core):
  - Edges never cross dialogues, so the RGCN scatter/gather runs as dense
    per-dialogue 0/1-adjacency matmuls.
  - All heavy GEMMs run in fp8 e4m3 with MatmulPerfMode.DoubleRow (two
    128-deep contraction subtiles per instruction = 2x bf16 throughput).
    Small weights are pre-scaled by powers of two on the host so their
    values sit in e4m3's normal range; the descales fold into the ACT/DVE
    psum->sbuf evacuations and the host-precomputed 1/deg vector.
  - The PE sequencer dispatch (~146ns per Ldweights+Matmult pair) costs
    more than most fp8 matmuls execute, so the emission minimizes PE
    instruction count: matmuls span two dialogues (512-wide moving APs)
    wherever the math allows (root, Xc, G^T), M is padded to 10 feature
    tiles so MEM contractions are exactly 5 DoubleRow pairs, and all
    transposes run as SBUF-to-SBUF DMA transposes on the SP queue instead
    of PE instructions.
  - tanh and exp live in the same ACT function table; the single ln at
    the end runs as ONE batched activation over all 8 row-sum vectors so
    the scheduler cannot interleave it with tanh/exp (which would thrash
    1.28us table loads).
  - The matchatt/softmax/classifier chain is emitted per-dialogue so
    dialogue d's softmax (ACT/DVE) overlaps dialogue d+1's Xc/scores
    matmuls (PE).

kernel(**inputs) takes FULL inputs, runs 8-core SPMD via
bass_utils.run_bass_kernel_spmd, returns the FULL (8192, 7) f32 output.
"""

import numpy as np
import ml_dtypes

BF16 = ml_dtypes.bfloat16
FP8 = ml_dtypes.float8_e4m3

# Problem constants (hardcoded per contract)
B, L, D, H, R, NB, C = 32, 256, 1024, 128, 8, 30, 7
MEM = D + H            # 1152
N = B * L              # 8192
NCORES = 8
DPC = B // NCORES      # dialogues per core = 4
NLOC = DPC * L         # nodes per core = 1024
NT = NLOC // 128       # node tiles per core = 8
KT = D // 128          # contraction tiles over D = 8
MT = MEM // 128        # tiles over MEM = 9
MTP = 10               # padded (even) feature tiles over MEM

# power-of-two pre-scales applied host-side before fp8 casts
S_WREL = 256.0         # w_rel entries ~2e-3: lift into e4m3 normal range
S_W = 32.0             # w_root1 / w_rel2 / w_root2 / w_t / w_lin (~2e-2)
S_ALF = 64.0           # alpha ~4e-3: lift out of e4m3 subnormals

_cache = {}


def _build_program(use_mask, biases_zero, warmup=24):
    import concourse.bacc as bacc
    import concourse.tile as tile
    import concourse.mybir as mybir
    import concourse.bass as bass

    dt = mybir.dt
    f32, bf16, fp8 = dt.float32, dt.bfloat16, dt.float8e4
    AX = mybir.AxisListType.X
    AF = mybir.ActivationFunctionType
    OP = mybir.AluOpType
    DR = mybir.MatmulPerfMode.DoubleRow

    nc = bacc.Bacc("TRN2", target_bir_lowering=False, debug=False,
                   num_devices=NCORES)

    dram = nc.dram_tensor
    xt_d = dram("xt", [D, NLOC], fp8, kind="ExternalInput")          # x^T
    wrel_d = dram("wrel", [D, R * H], fp8, kind="ExternalInput")     # *S_WREL
    wr1_d = dram("wr1", [D, H], fp8, kind="ExternalInput")           # *S_W
    at_d = dram("at", [DPC, R, L, L], fp8, kind="ExternalInput")     # A^T 0/1
    bt_d = dram("bt", [DPC, L, L], fp8, kind="ExternalInput")        # B^T 0/1
    invd_d = dram("invd", [DPC, L], bf16, kind="ExternalInput")      # 1/deg/S_WREL
    w2_d = dram("w2", [2, H, H], fp8, kind="ExternalInput")          # *S_W
    wt_d = dram("wt", [MTP * 128, MEM], fp8, kind="ExternalInput")   # *S_W, padded
    wlin_d = dram("wlin", [MTP * 128, H], fp8, kind="ExternalInput")  # *S_W, padded
    wfc_d = dram("wfc", [H, C], bf16, kind="ExternalInput")
    bias_d = dram("bias", [128, 13], f32, kind="ExternalInput")
    bfc_d = dram("bfc", [1, C], bf16, kind="ExternalInput")
    if use_mask:
        um_d = dram("um", [DPC, 2, L], f32, kind="ExternalInput")    # um^2, um
    out_d = dram("out", [NLOC, C], f32, kind="ExternalOutput")

    with tile.TileContext(nc) as tc:
        from contextlib import ExitStack
        with ExitStack() as ctx:
            consts = ctx.enter_context(tc.tile_pool(name="consts", bufs=1))
            big = ctx.enter_context(tc.tile_pool(name="big", bufs=1))
            work = ctx.enter_context(tc.tile_pool(name="work", bufs=6))
            ps = ctx.enter_context(tc.tile_pool(name="ps", bufs=6, space="PSUM"))
            pst = ctx.enter_context(tc.tile_pool(name="pst", bufs=2, space="PSUM"))

            dma_a = nc.sync.dma_start      # SP HWDGE: PE-critical operands
            dma_tr = nc.sync.dma_start_transpose
            dma_b = nc.gpsimd.dma_start    # SWDGE: everything else
            mm = nc.tensor.matmul

            # ---- persistent operand loads, ordered by first use ----
            wrel = consts.tile([128, KT, R, H], fp8)
            xt = consts.tile([128, KT, NLOC], fp8)
            dma_a(out=wrel[:, :, 0:4, :],
                  in_=wrel_d[:, 0:512].rearrange("(k p) n -> p k n", p=128))
            dma_a(out=xt[:, :, 0:512],
                  in_=xt_d[:, 0:512].rearrange("(k p) n -> p k n", p=128))
            dma_a(out=wrel[:, :, 4:8, :],
                  in_=wrel_d[:, 512:1024].rearrange("(k p) n -> p k n", p=128))
            dma_a(out=xt[:, :, 512:1024],
                  in_=xt_d[:, 512:1024].rearrange("(k p) n -> p k n", p=128))
            wr1 = consts.tile([128, KT, H], fp8)
            dma_a(out=wr1, in_=wr1_d[:].rearrange("(k p) n -> p k n", p=128))

            at = consts.tile([128, DPC, R, 2, L], fp8)
            dma_b(out=at,
                  in_=at_d[:].rearrange("d r (st p) t -> p d r st t", p=128))
            bt = consts.tile([128, DPC, 2, L], fp8)
            dma_b(out=bt, in_=bt_d[:].rearrange("d (st p) t -> p d st t", p=128))
            w2 = consts.tile([128, 2, H], fp8)
            dma_b(out=w2, in_=w2_d[:].rearrange("j p h -> p j h"))
            wt = consts.tile([128, MTP, MEM], fp8)
            dma_b(out=wt, in_=wt_d[:].rearrange("(m p) n -> p m n", p=128))
            wlin = consts.tile([128, MTP, H], fp8)
            dma_b(out=wlin, in_=wlin_d[:].rearrange("(m p) n -> p m n", p=128))
            wfc = consts.tile([128, C], bf16)
            dma_b(out=wfc, in_=wfc_d[:])
            bias = consts.tile([128, 13], f32)
            dma_b(out=bias, in_=bias_d[:])
            bfc = consts.tile([1, C], bf16)
            dma_b(out=bfc, in_=bfc_d[:])

            def bcast(dst, src_ap):
                bc = bass.AP(tensor=src_ap.tensor, offset=src_ap.offset,
                             ap=[[0, 128]] + list(src_ap.ap))
                nc.gpsimd.dma_start(out=dst, in_=bc)

            invd = consts.tile([128, DPC, L], bf16)
            bcast(invd, invd_d[:])
            if use_mask:
                um = consts.tile([128, DPC, 2, L], f32)
                bcast(um, um_d[:])

            ones_row = consts.tile([1, 128], bf16)
            nc.vector.memset(ones_row, 1.0)

            # zero pads so every MEM contraction runs as 5 DoubleRow pairs;
            # out2T slot 1 and XcT slot 9 stay zero.
            out2T = consts.tile([128, 2, NLOC], fp8)
            nc.vector.memset(out2T, 0.0)
            XcT = consts.tile([128, MTP, NLOC], fp8)
            nc.vector.memset(XcT[:, MT, :], 0.0)

            # hold the PE p-state clock up during the input-DMA lead-in;
            # `warm` psum is never read.
            warm_in = consts.tile([128, 128], bf16)
            nc.vector.memset(warm_in, 0.0)
            warm = ps.tile([128, 512], f32, tag="mm")
            for _ in range(warmup):
                mm(warm[:, :128], lhsT=warm_in, rhs=warm_in, start=True,
                   stop=True, skip_group_check=True)

            # ---- stage 1: xr[n, r, h] = x @ w_rel (all relations) ----
            # psum = S_WREL * true; stored at that scale in fp8.
            xr = consts.tile([128, NT, R, H], fp8)
            evac = [nc.vector, nc.scalar]
            for idx, (h2, i) in enumerate(
                    (h2, i) for h2 in range(2) for i in range(NT)):
                p = ps.tile([128, 512], f32, tag="mm")
                for kk in range(0, KT, 2):
                    mm(p, lhsT=xt[:, kk:kk + 2, i * 128:(i + 1) * 128],
                       rhs=wrel[:, kk:kk + 2, 4 * h2:4 * h2 + 4, :],
                       start=(kk == 0), stop=(kk == KT - 2), perf_mode=DR)
                eng = evac[idx % 2]
                if eng is nc.scalar:
                    eng.activation(xr[:, i, 4 * h2:4 * h2 + 4, :], p, AF.Identity)
                else:
                    eng.tensor_copy(xr[:, i, 4 * h2:4 * h2 + 4, :], p)

            out1T = consts.tile([128, DPC, L], bf16)   # [h, dlg, n]
            out1b = consts.tile([128, NT, H], bf16)    # [n, h] via DMA transpose
            out1 = consts.tile([128, NT, H], fp8)
            nbout = consts.tile([128, DPC, 2, L], fp8)
            hidT = consts.tile([128, DPC, L], bf16)

            # ---- stage 2+3: RGCN agg/root + GraphConv ----
            # root^T spans two dialogues per psum (512-wide moving rhs)
            prs = []
            for dg in range(2):
                pr = ps.tile([128, 512], f32, tag="mm")
                for kk in range(0, KT, 2):
                    mm(pr, lhsT=wr1[:, kk:kk + 2, :],
                       rhs=xt[:, kk:kk + 2, dg * 512:(dg + 1) * 512],
                       start=(kk == 0), stop=(kk == KT - 2), perf_mode=DR)
                prs.append(pr)
            for d in range(DPC):
                pa = ps.tile([128, 512], f32, tag="mm")
                for st in range(2):
                    for rp in range(0, R, 2):
                        mm(pa[:, :L], lhsT=xr[:, 2 * d + st, rp:rp + 2, :],
                           rhs=at[:, d, rp:rp + 2, st, :],
                           start=(st == 0 and rp == 0),
                           stop=(st == 1 and rp == R - 2),
                           perf_mode=DR, skip_group_check=True)
                agg = work.tile([128, L], f32, tag="agg")
                nc.vector.tensor_mul(agg, pa[:, :L], invd[:, d, :])
                # out1 = root/S_W + agg  (bias[:,0] = 1/S_W)
                pr = prs[d // 2][:, (d % 2) * L:(d % 2 + 1) * L]
                nc.vector.scalar_tensor_tensor(
                    out=out1T[:, d, :], in0=pr, scalar=bias[:, 0:1],
                    in1=agg, op0=OP.mult, op1=OP.add)
                if not biases_zero:
                    nc.vector.tensor_scalar_add(out1T[:, d, :], out1T[:, d, :],
                                                bias[:, 1:2])
                for st in range(2):
                    dma_tr(out=out1b[:, 2 * d + st, :],
                           in_=out1T[:, d, st * 128:(st + 1) * 128])
                nc.gpsimd.tensor_copy(out1[:, 2 * d:2 * d + 2, :],
                                      out1b[:, 2 * d:2 * d + 2, :])
                p2 = ps.tile([128, 512], f32, tag="mm")
                mm(p2[:, :L], lhsT=out1[:, 2 * d:2 * d + 2, :],
                   rhs=bt[:, d, 0:2, :], start=True, stop=True, perf_mode=DR,
                   skip_group_check=True)
                nc.vector.tensor_copy(nbout[:, d, 0, :], p2[:, :L])
                nc.gpsimd.tensor_copy(nbout[:, d, 1, :], out1T[:, d, :])
                p3 = ps.tile([128, 512], f32, tag="mm")
                mm(p3[:, :L], lhsT=w2[:, 0:2, :], rhs=nbout[:, d, 0:2, :],
                   start=True, stop=True, perf_mode=DR, skip_group_check=True)
                nc.scalar.activation(out2T[:, 0, d * L:(d + 1) * L], p3[:, :L],
                                     AF.Identity, scale=1.0 / S_W,
                                     bias=bias[:, 2:3])

            # M^T feature-tile pair accessors (5 DoubleRow pairs over MEM)
            def m_pair(mp, lo, width):
                if mp < 4:
                    return xt[:, 2 * mp:2 * mp + 2, lo:lo + width]
                return out2T[:, 0:2, lo:lo + width]

            # ---- stages 5-8 interleaved per dialogue ----
            zs, nmxs, alfs = {}, {}, {}
            alphaTb = consts.tile([128, DPC, 2, L], bf16)
            alphaT = consts.tile([128, DPC, 2, L], fp8)
            GTb = consts.tile([128, NLOC], bf16)       # G^T = wlin^T M^T
            G8b = consts.tile([128, NT, H], bf16)
            G8 = consts.tile([128, NT, H], fp8)
            o_all = consts.tile([128, DPC, 2, 8], f32)
            s7all = consts.tile([128, DPC * 2], f32)

            def emit_xc(dg):
                # Xc^T = (w_t^T M^T)/S_W + b_t over two dialogues at once
                for n2 in range(MT):
                    p4 = ps.tile([128, 512], f32, tag="mm")
                    for mp in range(5):
                        mm(p4, lhsT=wt[:, 2 * mp:2 * mp + 2,
                                       n2 * 128:(n2 + 1) * 128],
                           rhs=m_pair(mp, dg * 512, 512), start=(mp == 0),
                           stop=(mp == 4), perf_mode=DR)
                    eng = evac[n2 % 2]
                    if eng is nc.scalar:
                        eng.activation(XcT[:, n2, dg * 512:(dg + 1) * 512], p4,
                                       AF.Identity, scale=1.0 / S_W,
                                       bias=bias[:, 3 + n2:4 + n2])
                    else:
                        eng.tensor_scalar(
                            out=XcT[:, n2, dg * 512:(dg + 1) * 512], in0=p4,
                            scalar1=1.0 / S_W, scalar2=bias[:, 3 + n2:4 + n2],
                            op0=OP.mult, op1=OP.add)

            def emit_gt(dg):
                # G^T = (wlin^T M^T)/S_W over two dialogues at once
                pg = ps.tile([128, 512], f32, tag="mm")
                for mp in range(5):
                    mm(pg, lhsT=wlin[:, 2 * mp:2 * mp + 2, :],
                       rhs=m_pair(mp, dg * 512, 512), start=(mp == 0),
                       stop=(mp == 4), perf_mode=DR)
                nc.scalar.activation(GTb[:, dg * 512:(dg + 1) * 512], pg,
                                     AF.Identity, scale=1.0 / S_W)

            def emit_scores_softmax(d):
                for tt in range(2):
                    p5 = ps.tile([128, 512], f32, tag="mm")
                    for n2 in range(0, MTP, 2):
                        lhsT = XcT[:, n2:n2 + 2,
                                   d * L + tt * 128:d * L + (tt + 1) * 128]
                        rhs = (m_pair(n2 // 2, d * L, L) if n2 < 8
                               else out2T[:, 0:2, d * L:(d + 1) * L])
                        mm(p5[:, :L], lhsT=lhsT, rhs=rhs, start=(n2 == 0),
                           stop=(n2 == 8), perf_mode=DR)
                    z = big.tile([128, L], f32, tag=f"z{d}{tt}")
                    if use_mask:
                        nc.vector.tensor_mul(z, p5[:, :L], um[:, d, 0, :])
                        nc.scalar.activation(z, z, AF.Tanh)
                    else:
                        nc.scalar.activation(z, p5[:, :L], AF.Tanh)
                    nmx = work.tile([128, 1], f32, tag="nmx")
                    nc.vector.reduce_max(out=nmx, in_=z, axis=AX, negate=True)
                    zs[(d, tt)] = z
                    nmxs[(d, tt)] = nmx

            def emit_softmax2(d):
                # alf = S_ALF * softmax(tanh(scores)) in bf16, then alpha^T
                # via SBUF-to-SBUF DMA transpose (no PE instructions)
                for tt in range(2):
                    z, nmx = zs[(d, tt)], nmxs[(d, tt)]
                    ssum = work.tile([128, 1], f32, tag="ssum")
                    nc.scalar.activation(z, z, AF.Exp, bias=nmx, accum_out=ssum)
                    if use_mask:
                        nc.vector.tensor_mul(z, z, um[:, d, 1, :])
                        nc.vector.reduce_sum(out=ssum, in_=z, axis=AX)
                    rinv = work.tile([128, 1], f32, tag="rinv")
                    nc.vector.reciprocal(rinv, ssum)
                    nc.vector.tensor_scalar(out=rinv, in0=rinv, scalar1=S_ALF,
                                            scalar2=None, op0=OP.mult)
                    alf = big.tile([128, L], bf16, tag=f"alf{d}{tt}")
                    nc.vector.tensor_scalar_mul(alf, z, rinv)
                    alfs[(d, tt)] = alf
                    for st in range(2):
                        dma_tr(out=alphaTb[:, d, st, tt * 128:(tt + 1) * 128],
                               in_=alf[:, st * 128:(st + 1) * 128])
                nc.gpsimd.tensor_copy(alphaT[:, d, :, :], alphaTb[:, d, :, :])

            def emit_att_cls(d):
                # hid^T = relu(G^T alpha^T / S_ALF), logits, max/exp
                p7 = ps.tile([128, 512], f32, tag="mm")
                mm(p7[:, :L], lhsT=G8[:, 2 * d:2 * d + 2, :],
                   rhs=alphaT[:, d, 0:2, :], start=True, stop=True,
                   perf_mode=DR, skip_group_check=True)
                nc.scalar.activation(hidT[:, d, :], p7[:, :L], AF.Relu,
                                     scale=1.0 / S_ALF, bias=bias[:, 12:13])
                for tt in range(2):
                    p8 = ps.tile([128, 512], f32, tag="mm")
                    mm(p8[:, :C], lhsT=hidT[:, d, tt * 128:(tt + 1) * 128],
                       rhs=wfc, start=True, stop=biases_zero)
                    if not biases_zero:
                        mm(p8[:, :C], lhsT=ones_row, rhs=bfc, start=False,
                           stop=True)
                    nm7 = work.tile([128, 1], f32, tag=f"nm7_{d}{tt}")
                    nc.vector.reduce_max(out=nm7, in_=p8[:, :C], axis=AX,
                                         negate=True)
                    e7 = work.tile([128, 8], f32, tag="e7")
                    nc.scalar.activation(e7[:, :C], p8[:, :C], AF.Exp,
                                         bias=nm7,
                                         accum_out=s7all[:, 2 * d + tt:
                                                         2 * d + tt + 1])
                    nc.vector.tensor_scalar_add(o_all[:, d, tt, :C], p8[:, :C],
                                                nm7)

            for dg in range(2):
                emit_xc(dg)
                for d in (2 * dg, 2 * dg + 1):
                    emit_scores_softmax(d)
                    emit_softmax2(d)
                emit_gt(dg)
                if dg == 1:
                    # G^T complete for all dialogues: node-major via DMA
                    # transpose, then fp8 casts
                    for n in range(NT):
                        dma_tr(out=G8b[:, n, :],
                               in_=GTb[:, n * 128:(n + 1) * 128])
                    for d in range(DPC):
                        nc.gpsimd.tensor_copy(G8[:, 2 * d:2 * d + 2, :],
                                              G8b[:, 2 * d:2 * d + 2, :])
                    emit_att_cls(0)
                    emit_att_cls(1)
            emit_att_cls(2)
            emit_att_cls(3)

            # ---- final log-softmax correction ----
            # ONE batched ln over all 8 row sums: its input depends on every
            # stage-8 exp, so the scheduler cannot hoist it between tanh/exp
            # (one table switch total).
            ls7 = consts.tile([128, DPC * 2], f32)
            nc.scalar.activation(ls7, s7all, AF.Ln)
            for d in range(DPC):
                for tt in range(2):
                    nc.vector.tensor_scalar(
                        out=o_all[:, d, tt, :C], in0=o_all[:, d, tt, :C],
                        scalar1=ls7[:, 2 * d + tt:2 * d + tt + 1],
                        scalar2=None, op0=OP.subtract)
            dma_a(out=out_d[:].rearrange("(d tt p) c -> p d tt c", d=DPC, tt=2),
                  in_=o_all[:, :, :, 0:C])

    nc.compile()
    return nc


def prep_inputs(x, edge_src, edge_dst, edge_type, umask, basis, comp,
                w_root1, b1, w_rel2, b_rel2, w_root2, w_t, b_t,
                w_lin, b_lin, w_fc, b_fc):
    """Host-side sharding / layout prep. Returns (in_maps, use_mask,
    biases_zero)."""
    x = np.asarray(x, np.float32)
    src = np.asarray(edge_src, np.int64)
    dst = np.asarray(edge_dst, np.int64)
    ety = np.asarray(edge_type, np.int64)
    umask = np.asarray(umask, np.float32)
    basis = np.asarray(basis, np.float32)
    comp = np.asarray(comp, np.float32)

    # dialogue-locality of edges (guaranteed by the windowed construction)
    g_s = src // L
    assert np.array_equal(g_s, dst // L), "edges must stay within a dialogue"

    w_rel = np.einsum('rb,bdh->rdh', comp, basis)
    wrel_layout = np.ascontiguousarray(
        (w_rel * S_WREL).transpose(1, 0, 2).reshape(D, R * H)).astype(FP8)

    deg = np.bincount(dst, minlength=N).astype(np.float64)
    inv_deg = np.where(deg > 0, 1.0 / np.maximum(deg, 1), 0.0)
    invd2 = (inv_deg / S_WREL).astype(BF16)

    at_all = np.zeros((B, R, L, L), np.float32)   # [dlg, r, src, dst] 0/1
    ls, ld = src % L, dst % L
    np.add.at(at_all, (g_s, ety, ls, ld), 1.0)
    bt_all = np.zeros((B, L, L), np.float32)
    np.add.at(bt_all, (g_s, ls, ld), 1.0)

    use_mask = not bool(np.all(umask == 1.0))
    b1 = np.asarray(b1, np.float32)
    b_rel2 = np.asarray(b_rel2, np.float32)
    b_t = np.asarray(b_t, np.float32)
    b_lin = np.asarray(b_lin, np.float32)
    b_fc = np.asarray(b_fc, np.float32)
    biases_zero = bool(np.all(b1 == 0) and np.all(b_fc == 0))

    bias_pack = np.zeros((128, 13), np.float32)
    bias_pack[:, 0] = 1.0 / S_W
    bias_pack[:, 1] = b1
    bias_pack[:, 2] = b_rel2
    bias_pack[:, 3:12] = b_t.reshape(9, 128).T
    bias_pack[:, 12] = b_lin

    def pad10(w):
        out = np.zeros((MTP * 128, w.shape[1]), np.float32)
        out[:MEM] = w
        return out

    shared = {
        "wrel": wrel_layout,
        "wr1": (np.asarray(w_root1, np.float32) * S_W).astype(FP8),
        "w2": (np.stack([np.asarray(w_rel2, np.float32),
                         np.asarray(w_root2, np.float32)]) * S_W).astype(FP8),
        "wt": pad10(np.asarray(w_t, np.float32) * S_W).astype(FP8),
        "wlin": pad10(np.asarray(w_lin, np.float32) * S_W).astype(FP8),
        "wfc": np.asarray(w_fc, np.float32).astype(BF16),
        "bias": bias_pack,
        "bfc": b_fc.reshape(1, C).astype(BF16),
    }

    in_maps = []
    for c in range(NCORES):
        xl = x[c * NLOC:(c + 1) * NLOC]
        m = dict(shared)
        m["xt"] = np.ascontiguousarray(xl.T).astype(FP8)
        m["at"] = at_all[c * DPC:(c + 1) * DPC].astype(FP8)
        m["bt"] = bt_all[c * DPC:(c + 1) * DPC].astype(FP8)
        m["invd"] = invd2[c * NLOC:(c + 1) * NLOC].reshape(DPC, L)
        if use_mask:
            uml = umask[c * DPC:(c + 1) * DPC]   # (DPC, L)
            m["um"] = np.stack([uml * uml, uml], axis=1).astype(np.float32)
        in_maps.append(m)
    return in_maps, use_mask, biases_zero


_last_results = None


def kernel(**inputs):
    global _last_results
    from concourse.bass_utils import run_bass_kernel_spmd

    in_maps, use_mask, biases_zero = prep_inputs(**inputs)
    key = (use_mask, biases_zero)
    if key not in _cache:
        _cache[key] = _build_program(use_mask, biases_zero)
    nc = _cache[key]
    res = run_bass_kernel_spmd(nc, in_maps, core_ids=list(range(NCORES)))
    _last_results = res
    return np.concatenate([res.results[c]["out"] for c in range(NCORES)],
                          axis=0)


# revision 8
# speedup vs baseline: 1.3064x; 1.3064x over previous
"""Trainium2 Bass kernel for nn_DialogueGCNModel (DialogueGCN forward).

Strategy (data-parallel over dialogues, 4 dialogues per core):
  - Edges never cross dialogues, so the RGCN scatter/gather runs as dense
    per-dialogue 0/1-adjacency matmuls.
  - All heavy GEMMs run in fp8 e4m3 with MatmulPerfMode.DoubleRow (two
    128-deep contraction subtiles per instruction = 2x bf16 throughput).
    Small weights are pre-scaled by powers of two on the host so their
    values sit in e4m3's normal range; the descales fold into the ACT/DVE
    psum->sbuf evacuations and the host-precomputed 1/deg vector.
  - The PE sequencer dispatch (~142ns per Ldweights+Matmult pair) costs
    more than most fp8 matmuls execute, so the emission minimizes PE
    instruction count: matmuls span two dialogues (512-wide moving APs)
    wherever the math allows (root, Xc, G^T), and M is padded to 10
    feature tiles so MEM contractions are exactly 5 DoubleRow pairs.
  - Every operand ships in partition-major host layout so each input DMA
    is 128 contiguous-per-partition descriptors (SWDGE descriptor count,
    not bytes, set the queue dispatch cost).
  - tanh and exp live in the same ACT function table; the single ln at
    the end runs as ONE batched activation over all 8 row-sum vectors so
    the scheduler cannot interleave it with tanh/exp (which would thrash
    1.28us table loads).
  - The matchatt/softmax/classifier chain is emitted per-dialogue so
    dialogue d's softmax (ACT/DVE) overlaps dialogue d+1's Xc/scores
    matmuls (PE).

kernel(**inputs) takes FULL inputs, runs 8-core SPMD via
bass_utils.run_bass_kernel_spmd, returns the FULL (8192, 7) f32 output.
"""

import numpy as np
import ml_dtypes

BF16 = ml_dtypes.bfloat16
FP8 = ml_dtypes.float8_e4m3

# Problem constants (hardcoded per contract)
B, L, D, H, R, NB, C = 32, 256, 1024, 128, 8, 30, 7
MEM = D + H            # 1152
N = B * L              # 8192
NCORES = 8
DPC = B // NCORES      # dialogues per core = 4
NLOC = DPC * L         # nodes per core = 1024
NT = NLOC // 128       # node tiles per core = 8
KT = D // 128          # contraction tiles over D = 8
MT = MEM // 128        # tiles over MEM = 9
MTP = 10               # padded (even) feature tiles over MEM

# power-of-two pre-scales applied host-side before fp8 casts
S_WREL = 256.0         # w_rel entries ~2e-3: lift into e4m3 normal range
S_W = 32.0             # w_root1 / w_rel2 / w_root2 / w_t / w_lin (~2e-2)
S_ALF = 64.0           # alpha ~4e-3: lift out of e4m3 subnormals

_cache = {}


def _build_program(use_mask, biases_zero, warmup=24):
    import concourse.bacc as bacc
    import concourse.tile as tile
    import concourse.mybir as mybir
    import concourse.bass as bass
    from concourse.masks import make_identity

    dt = mybir.dt
    f32, bf16, fp8 = dt.float32, dt.bfloat16, dt.float8e4
    AX = mybir.AxisListType.X
    AF = mybir.ActivationFunctionType
    OP = mybir.AluOpType
    DR = mybir.MatmulPerfMode.DoubleRow

    nc = bacc.Bacc("TRN2", target_bir_lowering=False, debug=False,
                   num_devices=NCORES)

    # all inputs ship pre-transposed to partition-major [128, ...] layout
    dram = nc.dram_tensor
    xt_d = dram("xt", [128, KT, NLOC], fp8, kind="ExternalInput")
    wrel_d = dram("wrel", [128, KT, R * H], fp8, kind="ExternalInput")
    wr1_d = dram("wr1", [128, KT, H], fp8, kind="ExternalInput")
    at_d = dram("at", [128, DPC, R, 2, L], fp8, kind="ExternalInput")
    bt_d = dram("bt", [128, DPC, 2, L], fp8, kind="ExternalInput")
    invd_d = dram("invd", [DPC, L], bf16, kind="ExternalInput")
    w2_d = dram("w2", [128, 2, H], fp8, kind="ExternalInput")
    wt_d = dram("wt", [128, MTP, MEM], fp8, kind="ExternalInput")
    wlin_d = dram("wlin", [128, MTP, H], fp8, kind="ExternalInput")
    wfc_d = dram("wfc", [H, C], bf16, kind="ExternalInput")
    bias_d = dram("bias", [128, 13], f32, kind="ExternalInput")
    bfc_d = dram("bfc", [1, C], bf16, kind="ExternalInput")
    if use_mask:
        um_d = dram("um", [DPC, 2, L], f32, kind="ExternalInput")    # um^2, um
    out_d = dram("out", [NLOC, C], f32, kind="ExternalOutput")

    with tile.TileContext(nc) as tc:
        from contextlib import ExitStack
        with ExitStack() as ctx:
            consts = ctx.enter_context(tc.tile_pool(name="consts", bufs=1))
            big = ctx.enter_context(tc.tile_pool(name="big", bufs=1))
            work = ctx.enter_context(tc.tile_pool(name="work", bufs=6))
            ps = ctx.enter_context(tc.tile_pool(name="ps", bufs=6, space="PSUM"))
            pst = ctx.enter_context(tc.tile_pool(name="pst", bufs=2, space="PSUM"))

            dma_a = nc.sync.dma_start      # SP HWDGE: PE-critical operands
            dma_b = nc.gpsimd.dma_start    # SWDGE: everything else
            mm = nc.tensor.matmul

            # ---- persistent operand loads, ordered by first use ----
            wrel = consts.tile([128, KT, R, H], fp8)
            xt = consts.tile([128, KT, NLOC], fp8)
            dma_a(out=wrel[:, :, 0:4, :], in_=wrel_d[:, :, 0:512])
            dma_a(out=xt[:, :, 0:512], in_=xt_d[:, :, 0:512])
            dma_a(out=wrel[:, :, 4:8, :], in_=wrel_d[:, :, 512:1024])
            dma_a(out=xt[:, :, 512:1024], in_=xt_d[:, :, 512:1024])
            wr1 = consts.tile([128, KT, H], fp8)
            dma_a(out=wr1, in_=wr1_d[:])

            at = consts.tile([128, DPC, R, 2, L], fp8)
            dma_b(out=at, in_=at_d[:])
            bt = consts.tile([128, DPC, 2, L], fp8)
            dma_b(out=bt, in_=bt_d[:])
            w2 = consts.tile([128, 2, H], fp8)
            dma_b(out=w2, in_=w2_d[:])
            wt = consts.tile([128, MTP, MEM], fp8)
            dma_b(out=wt, in_=wt_d[:])
            wlin = consts.tile([128, MTP, H], fp8)
            dma_b(out=wlin, in_=wlin_d[:])
            wfc = consts.tile([128, C], bf16)
            dma_b(out=wfc, in_=wfc_d[:])
            bias = consts.tile([128, 13], f32)
            dma_b(out=bias, in_=bias_d[:])
            bfc = consts.tile([1, C], bf16)
            dma_b(out=bfc, in_=bfc_d[:])

            def bcast(dst, src_ap):
                bc = bass.AP(tensor=src_ap.tensor, offset=src_ap.offset,
                             ap=[[0, 128]] + list(src_ap.ap))
                nc.gpsimd.dma_start(out=dst, in_=bc)

            invd = consts.tile([128, DPC, L], bf16)
            bcast(invd, invd_d[:])
            if use_mask:
                um = consts.tile([128, DPC, 2, L], f32)
                bcast(um, um_d[:])

            # warm-up operand first so the PE can start immediately
            warm_in = consts.tile([128, 128], bf16)
            nc.vector.memset(warm_in, 0.0)
            warm = ps.tile([128, 512], f32, tag="mm")
            for _ in range(warmup):
                mm(warm[:, :128], lhsT=warm_in, rhs=warm_in, start=True,
                   stop=True, skip_group_check=True)

            ones_row = consts.tile([1, 128], bf16)
            nc.vector.memset(ones_row, 1.0)
            ident = consts.tile([128, 128], bf16)
            make_identity(nc, ident)

            # zero pads so every MEM contraction runs as 5 DoubleRow pairs;
            # out2T slot 1 and XcT slot 9 stay zero.
            out2T = consts.tile([128, 2, NLOC], fp8)
            nc.gpsimd.memset(out2T[:, 1, :], 0.0)
            XcT = consts.tile([128, MTP, NLOC], fp8)
            nc.gpsimd.memset(XcT[:, MT, :], 0.0)

            # ---- stage 1: xr[n, r, h] = x @ w_rel (all relations) ----
            # psum = S_WREL * true; stored at that scale in fp8.
            xr = consts.tile([128, NT, R, H], fp8)
            evac = [nc.vector, nc.scalar]
            for idx, (h2, i) in enumerate(
                    (h2, i) for h2 in range(2) for i in range(NT)):
                p = ps.tile([128, 512], f32, tag="mm")
                for kk in range(0, KT, 2):
                    mm(p, lhsT=xt[:, kk:kk + 2, i * 128:(i + 1) * 128],
                       rhs=wrel[:, kk:kk + 2, 4 * h2:4 * h2 + 4, :],
                       start=(kk == 0), stop=(kk == KT - 2), perf_mode=DR)
                eng = evac[idx % 2]
                if eng is nc.scalar:
                    eng.activation(xr[:, i, 4 * h2:4 * h2 + 4, :], p, AF.Identity)
                else:
                    eng.tensor_copy(xr[:, i, 4 * h2:4 * h2 + 4, :], p)

            out1T = consts.tile([128, DPC, L], bf16)   # [h, dlg, n]
            out1 = consts.tile([128, NT, H], fp8)      # [n, h]
            nbout = consts.tile([128, DPC, 2, L], fp8)
            hidT = consts.tile([128, DPC, L], bf16)

            # ---- stage 2+3: RGCN agg/root + GraphConv ----
            # root^T spans two dialogues per psum (512-wide moving rhs)
            prs = []
            for dg in range(2):
                pr = ps.tile([128, 512], f32, tag="mm")
                for kk in range(0, KT, 2):
                    mm(pr, lhsT=wr1[:, kk:kk + 2, :],
                       rhs=xt[:, kk:kk + 2, dg * 512:(dg + 1) * 512],
                       start=(kk == 0), stop=(kk == KT - 2), perf_mode=DR)
                prs.append(pr)
            for d in range(DPC):
                pa = ps.tile([128, 512], f32, tag="mm")
                for st in range(2):
                    for rp in range(0, R, 2):
                        mm(pa[:, :L], lhsT=xr[:, 2 * d + st, rp:rp + 2, :],
                           rhs=at[:, d, rp:rp + 2, st, :],
                           start=(st == 0 and rp == 0),
                           stop=(st == 1 and rp == R - 2),
                           perf_mode=DR, skip_group_check=True)
                agg = work.tile([128, L], f32, tag="agg")
                nc.vector.tensor_mul(agg, pa[:, :L], invd[:, d, :])
                # out1 = root/S_W + agg  (bias[:,0] = 1/S_W)
                pr = prs[d // 2][:, (d % 2) * L:(d % 2 + 1) * L]
                nc.vector.scalar_tensor_tensor(
                    out=out1T[:, d, :], in0=pr, scalar=bias[:, 0:1],
                    in1=agg, op0=OP.mult, op1=OP.add)
                if not biases_zero:
                    nc.vector.tensor_scalar_add(out1T[:, d, :], out1T[:, d, :],
                                                bias[:, 1:2])
                for st in range(2):
                    tp = pst.tile([128, 128], bf16, tag="tr")
                    nc.tensor.transpose(tp, out1T[:, d, st * 128:(st + 1) * 128],
                                        ident)
                    nc.vector.tensor_copy(out1[:, 2 * d + st, :], tp)
                p2 = ps.tile([128, 512], f32, tag="mm")
                mm(p2[:, :L], lhsT=out1[:, 2 * d:2 * d + 2, :],
                   rhs=bt[:, d, 0:2, :], start=True, stop=True, perf_mode=DR,
                   skip_group_check=True)
                nc.vector.tensor_copy(nbout[:, d, 0, :], p2[:, :L])
                nc.gpsimd.tensor_copy(nbout[:, d, 1, :], out1T[:, d, :])
                p3 = ps.tile([128, 512], f32, tag="mm")
                mm(p3[:, :L], lhsT=w2[:, 0:2, :], rhs=nbout[:, d, 0:2, :],
                   start=True, stop=True, perf_mode=DR, skip_group_check=True)
                nc.scalar.activation(out2T[:, 0, d * L:(d + 1) * L], p3[:, :L],
                                     AF.Identity, scale=1.0 / S_W,
                                     bias=bias[:, 2:3])

            # M^T feature-tile pair accessors (5 DoubleRow pairs over MEM)
            def m_pair(mp, lo, width):
                if mp < 4:
                    return xt[:, 2 * mp:2 * mp + 2, lo:lo + width]
                return out2T[:, 0:2, lo:lo + width]

            # ---- stages 5-8 interleaved per dialogue ----
            zs, nmxs, alfs = {}, {}, {}
            alphaT = consts.tile([128, DPC, 2, L], fp8)
            GTb = consts.tile([128, NLOC], bf16)       # G^T = wlin^T M^T
            G8 = consts.tile([128, NT, H], fp8)
            o_all = consts.tile([128, DPC, 2, 8], f32)
            s7all = consts.tile([128, DPC * 2], f32)

            def emit_xc(dg):
                # Xc^T = (w_t^T M^T)/S_W + b_t over two dialogues at once
                for n2 in range(MT):
                    p4 = ps.tile([128, 512], f32, tag="mm")
                    for mp in range(5):
                        mm(p4, lhsT=wt[:, 2 * mp:2 * mp + 2,
                                       n2 * 128:(n2 + 1) * 128],
                           rhs=m_pair(mp, dg * 512, 512), start=(mp == 0),
                           stop=(mp == 4), perf_mode=DR)
                    eng = evac[n2 % 2]
                    if eng is nc.scalar:
                        eng.activation(XcT[:, n2, dg * 512:(dg + 1) * 512], p4,
                                       AF.Identity, scale=1.0 / S_W,
                                       bias=bias[:, 3 + n2:4 + n2])
                    else:
                        eng.tensor_scalar(
                            out=XcT[:, n2, dg * 512:(dg + 1) * 512], in0=p4,
                            scalar1=1.0 / S_W, scalar2=bias[:, 3 + n2:4 + n2],
                            op0=OP.mult, op1=OP.add)

            def emit_gt(dg):
                # G^T = (wlin^T M^T)/S_W over two dialogues at once
                pg = ps.tile([128, 512], f32, tag="mm")
                for mp in range(5):
                    mm(pg, lhsT=wlin[:, 2 * mp:2 * mp + 2, :],
                       rhs=m_pair(mp, dg * 512, 512), start=(mp == 0),
                       stop=(mp == 4), perf_mode=DR)
                nc.scalar.activation(GTb[:, dg * 512:(dg + 1) * 512], pg,
                                     AF.Identity, scale=1.0 / S_W)

            def emit_scores_softmax(d):
                for tt in range(2):
                    p5 = ps.tile([128, 512], f32, tag="mm")
                    for n2 in range(0, MTP, 2):
                        lhsT = XcT[:, n2:n2 + 2,
                                   d * L + tt * 128:d * L + (tt + 1) * 128]
                        rhs = (m_pair(n2 // 2, d * L, L) if n2 < 8
                               else out2T[:, 0:2, d * L:(d + 1) * L])
                        mm(p5[:, :L], lhsT=lhsT, rhs=rhs, start=(n2 == 0),
                           stop=(n2 == 8), perf_mode=DR)
                    z = big.tile([128, L], f32, tag=f"z{d}{tt}")
                    if use_mask:
                        nc.vector.tensor_mul(z, p5[:, :L], um[:, d, 0, :])
                        nc.scalar.activation(z, z, AF.Tanh)
                    else:
                        nc.scalar.activation(z, p5[:, :L], AF.Tanh)
                    nmx = work.tile([128, 1], f32, tag="nmx")
                    nc.vector.reduce_max(out=nmx, in_=z, axis=AX, negate=True)
                    zs[(d, tt)] = z
                    nmxs[(d, tt)] = nmx

            def emit_softmax2(d):
                # alf = S_ALF * softmax(tanh(scores)) in bf16
                for tt in range(2):
                    z, nmx = zs[(d, tt)], nmxs[(d, tt)]
                    ssum = work.tile([128, 1], f32, tag="ssum")
                    nc.scalar.activation(z, z, AF.Exp, bias=nmx, accum_out=ssum)
                    if use_mask:
                        nc.vector.tensor_mul(z, z, um[:, d, 1, :])
                        nc.vector.reduce_sum(out=ssum, in_=z, axis=AX)
                    rinv = work.tile([128, 1], f32, tag="rinv")
                    nc.vector.reciprocal(rinv, ssum)
                    nc.vector.tensor_scalar(out=rinv, in0=rinv, scalar1=S_ALF,
                                            scalar2=None, op0=OP.mult)
                    alf = big.tile([128, L], bf16, tag=f"alf{d}{tt}")
                    nc.vector.tensor_scalar_mul(alf, z, rinv)
                    alfs[(d, tt)] = alf

            def emit_alpha_t(d):
                # alpha^T via PE transpose, fp8 cast on the psum copy
                for tt in range(2):
                    for st in range(2):
                        tp = pst.tile([128, 128], bf16, tag="tr")
                        nc.tensor.transpose(
                            tp, alfs[(d, tt)][:, st * 128:(st + 1) * 128], ident)
                        nc.vector.tensor_copy(
                            alphaT[:, d, st, tt * 128:(tt + 1) * 128], tp)

            def emit_att_cls(d):
                # hid^T = relu(G^T alpha^T / S_ALF), logits, max/exp
                p7 = ps.tile([128, 512], f32, tag="mm")
                mm(p7[:, :L], lhsT=G8[:, 2 * d:2 * d + 2, :],
                   rhs=alphaT[:, d, 0:2, :], start=True, stop=True,
                   perf_mode=DR, skip_group_check=True)
                nc.scalar.activation(hidT[:, d, :], p7[:, :L], AF.Relu,
                                     scale=1.0 / S_ALF, bias=bias[:, 12:13])
                for tt in range(2):
                    p8 = ps.tile([128, 512], f32, tag="mm")
                    mm(p8[:, :C], lhsT=hidT[:, d, tt * 128:(tt + 1) * 128],
                       rhs=wfc, start=True, stop=biases_zero)
                    if not biases_zero:
                        mm(p8[:, :C], lhsT=ones_row, rhs=bfc, start=False,
                           stop=True)
                    nm7 = work.tile([128, 1], f32, tag=f"nm7_{d}{tt}")
                    nc.vector.reduce_max(out=nm7, in_=p8[:, :C], axis=AX,
                                         negate=True)
                    e7 = work.tile([128, 8], f32, tag="e7")
                    nc.scalar.activation(e7[:, :C], p8[:, :C], AF.Exp,
                                         bias=nm7,
                                         accum_out=s7all[:, 2 * d + tt:
                                                         2 * d + tt + 1])
                    nc.vector.tensor_scalar_add(o_all[:, d, tt, :C], p8[:, :C],
                                                nm7)

            for dg in range(2):
                emit_xc(dg)
                for d in (2 * dg, 2 * dg + 1):
                    emit_scores_softmax(d)
                    emit_softmax2(d)
                    emit_alpha_t(d)
                emit_gt(dg)
                if dg == 1:
                    # G^T complete: node-major via PE transpose + fp8 cast
                    for n in range(NT):
                        tp = pst.tile([128, 128], bf16, tag="tr")
                        nc.tensor.transpose(tp, GTb[:, n * 128:(n + 1) * 128],
                                            ident)
                        nc.vector.tensor_copy(G8[:, n, :], tp)
                    emit_att_cls(0)
                    emit_att_cls(1)
            emit_att_cls(2)
            emit_att_cls(3)

            # ---- final log-softmax correction ----
            # ONE batched ln over all 8 row sums: its input depends on every
            # stage-8 exp, so the scheduler cannot hoist it between tanh/exp
            # (one table switch total).
            ls7 = consts.tile([128, DPC * 2], f32)
            nc.scalar.activation(ls7, s7all, AF.Ln)
            for d in range(DPC):
                for tt in range(2):
                    nc.vector.tensor_scalar(
                        out=o_all[:, d, tt, :C], in0=o_all[:, d, tt, :C],
                        scalar1=ls7[:, 2 * d + tt:2 * d + tt + 1],
                        scalar2=None, op0=OP.subtract)
            dma_a(out=out_d[:].rearrange("(d tt p) c -> p d tt c", d=DPC, tt=2),
                  in_=o_all[:, :, :, 0:C])

    nc.compile()
    return nc


def prep_inputs(x, edge_src, edge_dst, edge_type, umask, basis, comp,
                w_root1, b1, w_rel2, b_rel2, w_root2, w_t, b_t,
                w_lin, b_lin, w_fc, b_fc):
    """Host-side sharding / layout prep. Returns (in_maps, use_mask,
    biases_zero). All device operands are packed partition-major
    ([128, ...]) so every DMA is one contiguous run per partition."""
    x = np.asarray(x, np.float32)
    src = np.asarray(edge_src, np.int64)
    dst = np.asarray(edge_dst, np.int64)
    ety = np.asarray(edge_type, np.int64)
    umask = np.asarray(umask, np.float32)
    basis = np.asarray(basis, np.float32)
    comp = np.asarray(comp, np.float32)

    # dialogue-locality of edges (guaranteed by the windowed construction)
    g_s = src // L
    assert np.array_equal(g_s, dst // L), "edges must stay within a dialogue"

    def pkm(a, p=128):
        """[k*p, n...] -> [p, k, n...] partition-major pack."""
        return np.ascontiguousarray(
            a.reshape(-1, p, *a.shape[1:]).swapaxes(0, 1))

    w_rel = np.einsum('rb,bdh->rdh', comp, basis)
    wrel_layout = (w_rel * S_WREL).transpose(1, 0, 2).reshape(D, R * H)

    deg = np.bincount(dst, minlength=N).astype(np.float64)
    inv_deg = np.where(deg > 0, 1.0 / np.maximum(deg, 1), 0.0)
    invd2 = (inv_deg / S_WREL).astype(BF16)

    at_all = np.zeros((B, R, L, L), np.float32)   # [dlg, r, src, dst] 0/1
    ls, ld = src % L, dst % L
    np.add.at(at_all, (g_s, ety, ls, ld), 1.0)
    bt_all = np.zeros((B, L, L), np.float32)
    np.add.at(bt_all, (g_s, ls, ld), 1.0)

    use_mask = not bool(np.all(umask == 1.0))
    b1 = np.asarray(b1, np.float32)
    b_rel2 = np.asarray(b_rel2, np.float32)
    b_t = np.asarray(b_t, np.float32)
    b_lin = np.asarray(b_lin, np.float32)
    b_fc = np.asarray(b_fc, np.float32)
    biases_zero = bool(np.all(b1 == 0) and np.all(b_fc == 0))

    bias_pack = np.zeros((128, 13), np.float32)
    bias_pack[:, 0] = 1.0 / S_W
    bias_pack[:, 1] = b1
    bias_pack[:, 2] = b_rel2
    bias_pack[:, 3:12] = b_t.reshape(9, 128).T
    bias_pack[:, 12] = b_lin

    def pad10(w):
        out = np.zeros((MTP * 128, w.shape[1]), np.float32)
        out[:MEM] = w
        return out

    shared = {
        "wrel": pkm(wrel_layout).astype(FP8),
        "wr1": pkm(np.asarray(w_root1, np.float32) * S_W).astype(FP8),
        "w2": np.ascontiguousarray(
            (np.stack([np.asarray(w_rel2, np.float32),
                       np.asarray(w_root2, np.float32)]) * S_W)
            .swapaxes(0, 1)).astype(FP8),
        "wt": pkm(pad10(np.asarray(w_t, np.float32) * S_W)).astype(FP8),
        "wlin": pkm(pad10(np.asarray(w_lin, np.float32) * S_W)).astype(FP8),
        "wfc": np.asarray(w_fc, np.float32).astype(BF16),
        "bias": bias_pack,
        "bfc": b_fc.reshape(1, C).astype(BF16),
    }

    in_maps = []
    for c in range(NCORES):
        xl = x[c * NLOC:(c + 1) * NLOC]
        m = dict(shared)
        m["xt"] = pkm(np.ascontiguousarray(xl.T)).astype(FP8)
        # [d, r, (st p), t] -> [p, d, r, st, t]
        m["at"] = np.ascontiguousarray(
            at_all[c * DPC:(c + 1) * DPC].reshape(DPC, R, 2, 128, L)
            .transpose(3, 0, 1, 2, 4)).astype(FP8)
        m["bt"] = np.ascontiguousarray(
            bt_all[c * DPC:(c + 1) * DPC].reshape(DPC, 2, 128, L)
            .transpose(2, 0, 1, 3)).astype(FP8)
        m["invd"] = invd2[c * NLOC:(c + 1) * NLOC].reshape(DPC, L)
        if use_mask:
            uml = umask[c * DPC:(c + 1) * DPC]   # (DPC, L)
            m["um"] = np.stack([uml * uml, uml], axis=1).astype(np.float32)
        in_maps.append(m)
    return in_maps, use_mask, biases_zero


_last_results = None


def kernel(**inputs):
    global _last_results
    from concourse.bass_utils import run_bass_kernel_spmd

    in_maps, use_mask, biases_zero = prep_inputs(**inputs)
    key = (use_mask, biases_zero)
    if key not in _cache:
        _cache[key] = _build_program(use_mask, biases_zero)
    nc = _cache[key]
    res = run_bass_kernel_spmd(nc, in_maps, core_ids=list(range(NCORES)))
    _last_results = res
    return np.concatenate([res.results[c]["out"] for c in range(NCORES)],
                          axis=0)


# revision 13
# speedup vs baseline: 1.4817x; 1.1342x over previous
"""Trainium2 Bass kernel for nn_DialogueGCNModel (DialogueGCN forward).

Strategy (data-parallel over dialogues, 4 dialogues per core):
  - Edges never cross dialogues, so the RGCN scatter/gather runs as dense
    per-dialogue 0/1-adjacency matmuls.
  - All heavy GEMMs run in fp8 e4m3 with MatmulPerfMode.DoubleRow (two
    128-deep contraction subtiles per instruction = 2x bf16 throughput).
    Small weights are pre-scaled by powers of two on the host so their
    values sit in e4m3's normal range; the descales fold into the ACT/DVE
    psum->sbuf evacuations and the host-precomputed 1/deg vector.
  - The PE sequencer dispatch (~142ns per Ldweights+Matmult pair) costs
    more than most fp8 matmuls execute, so the emission minimizes PE
    instruction count: matmuls span two dialogues (512-wide moving APs)
    wherever the math allows (root, Xc, G^T), and M is padded to 10
    feature tiles so MEM contractions are exactly 5 DoubleRow pairs.
  - Every operand ships in partition-major host layout so each input DMA
    is 128 contiguous-per-partition descriptors (SWDGE descriptor count,
    not bytes, set the queue dispatch cost).
  - tanh and exp live in the same ACT function table; the single ln at
    the end runs as ONE batched activation over all 8 row-sum vectors so
    the scheduler cannot interleave it with tanh/exp (which would thrash
    1.28us table loads).
  - The matchatt/softmax/classifier chain is emitted per-dialogue so
    dialogue d's softmax (ACT/DVE) overlaps dialogue d+1's Xc/scores
    matmuls (PE).

kernel(**inputs) takes FULL inputs, runs 8-core SPMD via
bass_utils.run_bass_kernel_spmd, returns the FULL (8192, 7) f32 output.
"""

import numpy as np
import ml_dtypes

BF16 = ml_dtypes.bfloat16
FP8 = ml_dtypes.float8_e4m3

# Problem constants (hardcoded per contract)
B, L, D, H, R, NB, C = 32, 256, 1024, 128, 8, 30, 7
MEM = D + H            # 1152
N = B * L              # 8192
NCORES = 8
DPC = B // NCORES      # dialogues per core = 4
NLOC = DPC * L         # nodes per core = 1024
NT = NLOC // 128       # node tiles per core = 8
KT = D // 128          # contraction tiles over D = 8
MT = MEM // 128        # tiles over MEM = 9
MTP = 10               # padded (even) feature tiles over MEM

# power-of-two pre-scales applied host-side before fp8 casts
S_WREL = 256.0         # w_rel entries ~2e-3: lift into e4m3 normal range
S_W = 32.0             # w_root1 / w_rel2 / w_root2 / w_t / w_lin (~2e-2)
S_ALF = 64.0           # alpha ~4e-3: lift out of e4m3 subnormals

_cache = {}


def _build_program(use_mask, biases_zero, warmup=24):
    import concourse.bacc as bacc
    import concourse.tile as tile
    import concourse.mybir as mybir
    import concourse.bass as bass
    from concourse.masks import make_identity

    dt = mybir.dt
    f32, bf16, fp8 = dt.float32, dt.bfloat16, dt.float8e4
    AX = mybir.AxisListType.X
    AF = mybir.ActivationFunctionType
    OP = mybir.AluOpType
    DR = mybir.MatmulPerfMode.DoubleRow

    nc = bacc.Bacc("TRN2", target_bir_lowering=False, debug=False,
                   num_devices=NCORES)

    # all inputs ship pre-transposed to partition-major [128, ...] layout
    dram = nc.dram_tensor
    xt_d = dram("xt", [128, KT, NLOC], fp8, kind="ExternalInput")
    wrel_d = dram("wrel", [128, KT, R * H], fp8, kind="ExternalInput")
    wr1_d = dram("wr1", [128, KT, H], fp8, kind="ExternalInput")
    at_d = dram("at", [128, DPC, R, 2, L], fp8, kind="ExternalInput")
    bt_d = dram("bt", [128, DPC, 2, L], fp8, kind="ExternalInput")
    invd_d = dram("invd", [DPC, L], bf16, kind="ExternalInput")
    w2_d = dram("w2", [128, 2, H], fp8, kind="ExternalInput")
    wt_d = dram("wt", [128, MTP, MEM], fp8, kind="ExternalInput")
    wlin_d = dram("wlin", [128, MTP, H], fp8, kind="ExternalInput")
    wfc_d = dram("wfc", [H, C], bf16, kind="ExternalInput")
    bias_d = dram("bias", [128, 13], f32, kind="ExternalInput")
    bfc_d = dram("bfc", [1, C], bf16, kind="ExternalInput")
    if use_mask:
        um_d = dram("um", [DPC, 2, L], f32, kind="ExternalInput")    # um^2, um
    out_d = dram("out", [NLOC, C], f32, kind="ExternalOutput")

    with tile.TileContext(nc) as tc:
        from contextlib import ExitStack
        with ExitStack() as ctx:
            consts = ctx.enter_context(tc.tile_pool(name="consts", bufs=1))
            big = ctx.enter_context(tc.tile_pool(name="big", bufs=1))
            work = ctx.enter_context(tc.tile_pool(name="work", bufs=6))
            ps = ctx.enter_context(tc.tile_pool(name="ps", bufs=6, space="PSUM"))
            pst = ctx.enter_context(tc.tile_pool(name="pst", bufs=2, space="PSUM"))

            dma_a = nc.sync.dma_start      # SP HWDGE: PE-critical operands
            dma_b = nc.gpsimd.dma_start    # SWDGE: everything else
            mm = nc.tensor.matmul

            # ---- persistent operand loads, ordered by first use ----
            wrel = consts.tile([128, KT, R, H], fp8)
            xt = consts.tile([128, KT, NLOC], fp8)
            dma_a(out=wrel[:, :, 0:4, :], in_=wrel_d[:, :, 0:512])
            dma_a(out=xt[:, :, 0:512], in_=xt_d[:, :, 0:512])
            dma_a(out=wrel[:, :, 4:8, :], in_=wrel_d[:, :, 512:1024])
            dma_a(out=xt[:, :, 512:1024], in_=xt_d[:, :, 512:1024])
            wr1 = consts.tile([128, KT, H], fp8)
            dma_a(out=wr1, in_=wr1_d[:])

            at = consts.tile([128, DPC, R, 2, L], fp8)
            dma_b(out=at, in_=at_d[:])
            bt = consts.tile([128, DPC, 2, L], fp8)
            dma_b(out=bt, in_=bt_d[:])
            w2 = consts.tile([128, 2, H], fp8)
            dma_b(out=w2, in_=w2_d[:])
            wt = consts.tile([128, MTP, MEM], fp8)
            dma_b(out=wt, in_=wt_d[:])
            wlin = consts.tile([128, MTP, H], fp8)
            dma_b(out=wlin, in_=wlin_d[:])
            wfc = consts.tile([128, C], bf16)
            dma_b(out=wfc, in_=wfc_d[:])
            bias = consts.tile([128, 13], f32)
            dma_b(out=bias, in_=bias_d[:])
            bfc = consts.tile([1, C], bf16)
            dma_b(out=bfc, in_=bfc_d[:])

            def bcast(dst, src_ap):
                bc = bass.AP(tensor=src_ap.tensor, offset=src_ap.offset,
                             ap=[[0, 128]] + list(src_ap.ap))
                nc.gpsimd.dma_start(out=dst, in_=bc)

            invd = consts.tile([128, DPC, L], bf16)
            bcast(invd, invd_d[:])
            if use_mask:
                um = consts.tile([128, DPC, 2, L], f32)
                bcast(um, um_d[:])

            # warm-up operand first so the PE can start immediately
            warm_in = consts.tile([128, 128], bf16)
            nc.vector.memset(warm_in, 0.0)
            warm = ps.tile([128, 512], f32, tag="mm")
            for _ in range(warmup):
                mm(warm[:, :128], lhsT=warm_in, rhs=warm_in, start=True,
                   stop=True, skip_group_check=True)

            ones_row = consts.tile([1, 128], bf16)
            nc.vector.memset(ones_row, 1.0)
            ident = consts.tile([128, 128], bf16)
            make_identity(nc, ident)

            # zero pads so every MEM contraction runs as 5 DoubleRow pairs;
            # out2T slot 1 and XcT slot 9 stay zero.
            out2T = consts.tile([128, 2, NLOC], fp8)
            nc.gpsimd.memset(out2T[:, 1, :], 0.0)
            XcT = consts.tile([128, MTP, NLOC], fp8)
            nc.gpsimd.memset(XcT[:, MT, :], 0.0)

            # ---- stage 1: xr[n, r, h] = x @ w_rel (all relations) ----
            # psum = S_WREL * true; stored at that scale in fp8.
            xr = consts.tile([128, NT, R, H], fp8)
            evac = [nc.vector, nc.scalar]
            for idx, (h2, i) in enumerate(
                    (h2, i) for h2 in range(2) for i in range(NT)):
                p = ps.tile([128, 512], f32, tag="mm")
                for kk in range(0, KT, 2):
                    mm(p, lhsT=xt[:, kk:kk + 2, i * 128:(i + 1) * 128],
                       rhs=wrel[:, kk:kk + 2, 4 * h2:4 * h2 + 4, :],
                       start=(kk == 0), stop=(kk == KT - 2), perf_mode=DR)
                eng = evac[idx % 2]
                if eng is nc.scalar:
                    eng.activation(xr[:, i, 4 * h2:4 * h2 + 4, :], p, AF.Identity)
                else:
                    eng.tensor_copy(xr[:, i, 4 * h2:4 * h2 + 4, :], p)

            out1T = consts.tile([128, DPC, L], bf16)   # [h, dlg, n]
            out1 = consts.tile([128, NT, H], fp8)      # [n, h]
            nbout = consts.tile([128, DPC, 2, L], fp8)
            hidT = consts.tile([128, DPC, L], bf16)

            # ---- stage 2+3: RGCN agg/root + GraphConv ----
            # root^T spans two dialogues per psum (512-wide moving rhs)
            prs = []
            for dg in range(2):
                pr = ps.tile([128, 512], f32, tag="mm")
                for kk in range(0, KT, 2):
                    mm(pr, lhsT=wr1[:, kk:kk + 2, :],
                       rhs=xt[:, kk:kk + 2, dg * 512:(dg + 1) * 512],
                       start=(kk == 0), stop=(kk == KT - 2), perf_mode=DR)
                prs.append(pr)
            # stage-major across dialogues so the PE never waits on one
            # dialogue's DVE chain
            for d in range(DPC):
                pa = ps.tile([128, 512], f32, tag="mm")
                for st in range(2):
                    for rp in range(0, R, 2):
                        mm(pa[:, :L], lhsT=xr[:, 2 * d + st, rp:rp + 2, :],
                           rhs=at[:, d, rp:rp + 2, st, :],
                           start=(st == 0 and rp == 0),
                           stop=(st == 1 and rp == R - 2),
                           perf_mode=DR, skip_group_check=True)
                agg = work.tile([128, L], f32, tag="agg")
                nc.vector.tensor_mul(agg, pa[:, :L], invd[:, d, :])
                # out1 = root/S_W + agg  (bias[:,0] = 1/S_W)
                pr = prs[d // 2][:, (d % 2) * L:(d % 2 + 1) * L]
                nc.vector.scalar_tensor_tensor(
                    out=out1T[:, d, :], in0=pr, scalar=bias[:, 0:1],
                    in1=agg, op0=OP.mult, op1=OP.add)
                if not biases_zero:
                    nc.vector.tensor_scalar_add(out1T[:, d, :], out1T[:, d, :],
                                                bias[:, 1:2])
                nc.gpsimd.tensor_copy(nbout[:, d, 1, :], out1T[:, d, :])
            for d in range(DPC):
                tp = pst.tile([128, 256], bf16, tag="tr")
                for st in range(2):
                    nc.tensor.transpose(tp[:, st * 128:(st + 1) * 128],
                                        out1T[:, d, st * 128:(st + 1) * 128],
                                        ident)
                nc.vector.tensor_copy(out1[:, 2 * d:2 * d + 2, :], tp)
            for d in range(DPC):
                p2 = ps.tile([128, 512], f32, tag="mm")
                mm(p2[:, :L], lhsT=out1[:, 2 * d:2 * d + 2, :],
                   rhs=bt[:, d, 0:2, :], start=True, stop=True, perf_mode=DR,
                   skip_group_check=True)
                nc.vector.tensor_copy(nbout[:, d, 0, :], p2[:, :L])
            for d in range(DPC):
                p3 = ps.tile([128, 512], f32, tag="mm")
                mm(p3[:, :L], lhsT=w2[:, 0:2, :], rhs=nbout[:, d, 0:2, :],
                   start=True, stop=True, perf_mode=DR, skip_group_check=True)
                nc.scalar.activation(out2T[:, 0, d * L:(d + 1) * L], p3[:, :L],
                                     AF.Identity, scale=1.0 / S_W,
                                     bias=bias[:, 2:3])

            # M^T feature-tile pair accessors (5 DoubleRow pairs over MEM)
            def m_pair(mp, lo, width):
                if mp < 4:
                    return xt[:, 2 * mp:2 * mp + 2, lo:lo + width]
                return out2T[:, 0:2, lo:lo + width]

            # ---- stages 5-8 interleaved per dialogue ----
            zs, nmxs, alfs = {}, {}, {}
            alphaT = consts.tile([128, DPC, 2, L], fp8)
            GTb = consts.tile([128, NLOC], bf16)       # G^T = wlin^T M^T
            G8 = consts.tile([128, NT, H], fp8)
            o_all = consts.tile([128, DPC, 2, 8], f32)
            nc.gpsimd.memset(o_all, 0.0)   # col 7 feeds a batched exp
            s7all = consts.tile([128, DPC * 2], f32)

            def emit_xc(dg):
                # Xc^T = (w_t^T M^T)/S_W + b_t over two dialogues at once
                for n2 in range(MT):
                    p4 = ps.tile([128, 512], f32, tag="mm")
                    for mp in range(5):
                        mm(p4, lhsT=wt[:, 2 * mp:2 * mp + 2,
                                       n2 * 128:(n2 + 1) * 128],
                           rhs=m_pair(mp, dg * 512, 512), start=(mp == 0),
                           stop=(mp == 4), perf_mode=DR)
                    eng = evac[n2 % 2]
                    if eng is nc.scalar:
                        eng.activation(XcT[:, n2, dg * 512:(dg + 1) * 512], p4,
                                       AF.Identity, scale=1.0 / S_W,
                                       bias=bias[:, 3 + n2:4 + n2])
                    else:
                        eng.tensor_scalar(
                            out=XcT[:, n2, dg * 512:(dg + 1) * 512], in0=p4,
                            scalar1=1.0 / S_W, scalar2=bias[:, 3 + n2:4 + n2],
                            op0=OP.mult, op1=OP.add)

            def emit_gt(dg):
                # G^T = (wlin^T M^T)/S_W over two dialogues at once
                pg = ps.tile([128, 512], f32, tag="mm")
                for mp in range(5):
                    mm(pg, lhsT=wlin[:, 2 * mp:2 * mp + 2, :],
                       rhs=m_pair(mp, dg * 512, 512), start=(mp == 0),
                       stop=(mp == 4), perf_mode=DR)
                nc.scalar.activation(GTb[:, dg * 512:(dg + 1) * 512], pg,
                                     AF.Identity, scale=1.0 / S_W)

            def emit_scores_softmax(d):
                # scores for both t-blocks, then ONE tanh over [128, 2L].
                # tanh output is bounded in [-1,1], so the softmax runs
                # without max-subtraction (shift-invariant, exp(|z|<=1)).
                z = big.tile([128, 2, L], f32, tag=f"z{d}")
                for tt in range(2):
                    p5 = ps.tile([128, 512], f32, tag="mm")
                    for n2 in range(0, MTP, 2):
                        lhsT = XcT[:, n2:n2 + 2,
                                   d * L + tt * 128:d * L + (tt + 1) * 128]
                        rhs = (m_pair(n2 // 2, d * L, L) if n2 < 8
                               else out2T[:, 0:2, d * L:(d + 1) * L])
                        mm(p5[:, :L], lhsT=lhsT, rhs=rhs, start=(n2 == 0),
                           stop=(n2 == 8), perf_mode=DR)
                    if use_mask:
                        nc.vector.tensor_mul(z[:, tt, :], p5[:, :L],
                                             um[:, d, 0, :])
                    else:
                        nc.vector.tensor_copy(z[:, tt, :], p5[:, :L])
                nc.scalar.activation(z, z, AF.Tanh)
                zs[d] = z

            def emit_softmax2(d):
                # alf = S_ALF * softmax(tanh(scores)) in bf16
                z = zs[d]
                for tt in range(2):
                    ssum = work.tile([128, 1], f32, tag="ssum")
                    nc.scalar.activation(z[:, tt, :], z[:, tt, :], AF.Exp,
                                         accum_out=ssum)
                    if use_mask:
                        nc.vector.tensor_mul(z[:, tt, :], z[:, tt, :],
                                             um[:, d, 1, :])
                        nc.vector.reduce_sum(out=ssum, in_=z[:, tt, :], axis=AX)
                    rinv = work.tile([128, 1], f32, tag="rinv")
                    nc.vector.reciprocal(rinv, ssum)
                    nc.vector.tensor_scalar(out=rinv, in0=rinv, scalar1=S_ALF,
                                            scalar2=None, op0=OP.mult)
                    alf = big.tile([128, L], bf16, tag=f"alf{d}{tt}")
                    nc.vector.tensor_scalar_mul(alf, z[:, tt, :], rinv)
                    alfs[(d, tt)] = alf

            def emit_alpha_t(d):
                # alpha^T via PE transpose; one paired evac per s-block
                for st in range(2):
                    tp = pst.tile([128, 256], bf16, tag="tr")
                    for tt in range(2):
                        nc.tensor.transpose(
                            tp[:, tt * 128:(tt + 1) * 128],
                            alfs[(d, tt)][:, st * 128:(st + 1) * 128], ident)
                    nc.vector.tensor_copy(alphaT[:, d, st, :], tp)

            def emit_att_cls(d):
                # hid^T = relu(G^T alpha^T / S_ALF), raw logits into o_all
                # (no max-subtraction: logits are O(0.1))
                p7 = ps.tile([128, 512], f32, tag="mm")
                mm(p7[:, :L], lhsT=G8[:, 2 * d:2 * d + 2, :],
                   rhs=alphaT[:, d, 0:2, :], start=True, stop=True,
                   perf_mode=DR, skip_group_check=True)
                nc.scalar.activation(hidT[:, d, :], p7[:, :L], AF.Relu,
                                     scale=1.0 / S_ALF, bias=bias[:, 12:13])
                for tt in range(2):
                    p8 = ps.tile([128, 512], f32, tag="mm")
                    mm(p8[:, :C], lhsT=hidT[:, d, tt * 128:(tt + 1) * 128],
                       rhs=wfc, start=True, stop=biases_zero)
                    if not biases_zero:
                        mm(p8[:, :C], lhsT=ones_row, rhs=bfc, start=False,
                           stop=True)
                    nc.vector.tensor_copy(o_all[:, d, tt, :C], p8[:, :C])

            for dg in range(2):
                emit_xc(dg)
                emit_gt(dg)
                for d in (2 * dg, 2 * dg + 1):
                    emit_scores_softmax(d)
                    emit_softmax2(d)
                    emit_alpha_t(d)
                if dg == 1:
                    # G^T complete: node-major via PE transpose + fp8 cast,
                    # one paired evac per two node tiles
                    for n in range(0, NT, 2):
                        tp = pst.tile([128, 256], bf16, tag="tr")
                        for j in range(2):
                            nc.tensor.transpose(
                                tp[:, j * 128:(j + 1) * 128],
                                GTb[:, (n + j) * 128:(n + j + 1) * 128], ident)
                        nc.vector.tensor_copy(G8[:, n:n + 2, :], tp)
                    emit_att_cls(0)
                    emit_att_cls(1)
            emit_att_cls(2)
            emit_att_cls(3)

            # ---- final log-softmax ----
            # ONE batched exp over all raw logits, per-slot sums, ONE
            # batched ln: the ln's input depends on every logit, so the
            # scheduler cannot hoist it between tanh/exp (one table switch
            # total).
            e_all = consts.tile([128, DPC, 2, 8], f32)
            nc.scalar.activation(e_all, o_all, AF.Exp)
            for d in range(DPC):
                for tt in range(2):
                    nc.vector.reduce_sum(
                        out=s7all[:, 2 * d + tt:2 * d + tt + 1],
                        in_=e_all[:, d, tt, :C], axis=AX)
            ls7 = consts.tile([128, DPC * 2], f32)
            nc.scalar.activation(ls7, s7all, AF.Ln)
            for d in range(DPC):
                for tt in range(2):
                    nc.vector.tensor_scalar(
                        out=o_all[:, d, tt, :C], in0=o_all[:, d, tt, :C],
                        scalar1=ls7[:, 2 * d + tt:2 * d + tt + 1],
                        scalar2=None, op0=OP.subtract)
            dma_a(out=out_d[:].rearrange("(d tt p) c -> p d tt c", d=DPC, tt=2),
                  in_=o_all[:, :, :, 0:C])

    nc.compile()
    return nc


def prep_inputs(x, edge_src, edge_dst, edge_type, umask, basis, comp,
                w_root1, b1, w_rel2, b_rel2, w_root2, w_t, b_t,
                w_lin, b_lin, w_fc, b_fc):
    """Host-side sharding / layout prep. Returns (in_maps, use_mask,
    biases_zero). All device operands are packed partition-major
    ([128, ...]) so every DMA is one contiguous run per partition."""
    x = np.asarray(x, np.float32)
    src = np.asarray(edge_src, np.int64)
    dst = np.asarray(edge_dst, np.int64)
    ety = np.asarray(edge_type, np.int64)
    umask = np.asarray(umask, np.float32)
    basis = np.asarray(basis, np.float32)
    comp = np.asarray(comp, np.float32)

    # dialogue-locality of edges (guaranteed by the windowed construction)
    g_s = src // L
    assert np.array_equal(g_s, dst // L), "edges must stay within a dialogue"

    def pkm(a, p=128):
        """[k*p, n...] -> [p, k, n...] partition-major pack."""
        return np.ascontiguousarray(
            a.reshape(-1, p, *a.shape[1:]).swapaxes(0, 1))

    w_rel = np.einsum('rb,bdh->rdh', comp, basis)
    wrel_layout = (w_rel * S_WREL).transpose(1, 0, 2).reshape(D, R * H)

    deg = np.bincount(dst, minlength=N).astype(np.float64)
    inv_deg = np.where(deg > 0, 1.0 / np.maximum(deg, 1), 0.0)
    invd2 = (inv_deg / S_WREL).astype(BF16)

    at_all = np.zeros((B, R, L, L), np.float32)   # [dlg, r, src, dst] 0/1
    ls, ld = src % L, dst % L
    np.add.at(at_all, (g_s, ety, ls, ld), 1.0)
    bt_all = np.zeros((B, L, L), np.float32)
    np.add.at(bt_all, (g_s, ls, ld), 1.0)

    use_mask = not bool(np.all(umask == 1.0))
    b1 = np.asarray(b1, np.float32)
    b_rel2 = np.asarray(b_rel2, np.float32)
    b_t = np.asarray(b_t, np.float32)
    b_lin = np.asarray(b_lin, np.float32)
    b_fc = np.asarray(b_fc, np.float32)
    biases_zero = bool(np.all(b1 == 0) and np.all(b_fc == 0))

    bias_pack = np.zeros((128, 13), np.float32)
    bias_pack[:, 0] = 1.0 / S_W
    bias_pack[:, 1] = b1
    bias_pack[:, 2] = b_rel2
    bias_pack[:, 3:12] = b_t.reshape(9, 128).T
    bias_pack[:, 12] = b_lin

    def pad10(w):
        out = np.zeros((MTP * 128, w.shape[1]), np.float32)
        out[:MEM] = w
        return out

    shared = {
        "wrel": pkm(wrel_layout).astype(FP8),
        "wr1": pkm(np.asarray(w_root1, np.float32) * S_W).astype(FP8),
        "w2": np.ascontiguousarray(
            (np.stack([np.asarray(w_rel2, np.float32),
                       np.asarray(w_root2, np.float32)]) * S_W)
            .swapaxes(0, 1)).astype(FP8),
        "wt": pkm(pad10(np.asarray(w_t, np.float32) * S_W)).astype(FP8),
        "wlin": pkm(pad10(np.asarray(w_lin, np.float32) * S_W)).astype(FP8),
        "wfc": np.asarray(w_fc, np.float32).astype(BF16),
        "bias": bias_pack,
        "bfc": b_fc.reshape(1, C).astype(BF16),
    }

    in_maps = []
    for c in range(NCORES):
        xl = x[c * NLOC:(c + 1) * NLOC]
        m = dict(shared)
        m["xt"] = pkm(np.ascontiguousarray(xl.T)).astype(FP8)
        # [d, r, (st p), t] -> [p, d, r, st, t]
        m["at"] = np.ascontiguousarray(
            at_all[c * DPC:(c + 1) * DPC].reshape(DPC, R, 2, 128, L)
            .transpose(3, 0, 1, 2, 4)).astype(FP8)
        m["bt"] = np.ascontiguousarray(
            bt_all[c * DPC:(c + 1) * DPC].reshape(DPC, 2, 128, L)
            .transpose(2, 0, 1, 3)).astype(FP8)
        m["invd"] = invd2[c * NLOC:(c + 1) * NLOC].reshape(DPC, L)
        if use_mask:
            uml = umask[c * DPC:(c + 1) * DPC]   # (DPC, L)
            m["um"] = np.stack([uml * uml, uml], axis=1).astype(np.float32)
        in_maps.append(m)
    return in_maps, use_mask, biases_zero


_last_results = None


def kernel(**inputs):
    global _last_results
    from concourse.bass_utils import run_bass_kernel_spmd

    in_maps, use_mask, biases_zero = prep_inputs(**inputs)
    key = (use_mask, biases_zero)
    if key not in _cache:
        _cache[key] = _build_program(use_mask, biases_zero)
    nc = _cache[key]
    res = run_bass_kernel_spmd(nc, in_maps, core_ids=list(range(NCORES)))
    _last_results = res
    return np.concatenate([res.results[c]["out"] for c in range(NCORES)],
                          axis=0)
